# revision 42
# baseline (speedup 1.0000x reference)
"""GPT forward Bass kernel for nn_GPTModel_2534030705251 on 8 TRN2 NeuronCores.

Strategy: batch-replicated, zero collectives (collectives cost ~1.5 ms
each on this runtime, dwarfing all compute). rank = batch*4 + vocab_quarter:
cores 0-3 each run batch 0's full 1024-token forward, cores 4-7 batch 1;
the LM head is sharded over vocab quarters (12800 padded columns/core).
Activations live feature-major [D, tokens] in SBUF so every weight matrix
streams in natural [D_in, D_out] layout as the stationary matmul operand.
bf16 matmul operands, fp32 PSUM accumulation, bf16 residual stream.

Optimizations over the 3.55ms baseline (final ~3.0ms):
- LN scale/shift folded into the consuming weight matrices host-side
  (ln1_s rows into Wq/Wk/Wv, ln2_s into W1, fn_s into W_head; shifts
  become per-output biases, with the V-path shift folded into bo via
  softmax-row-sum=1). Kills all LN apply activations on the Scalar eng.
- bf16 residual stream: halves DVE element cost, kills the bf16 staging
  copies that LN stats needed, halves x DMA.
- rstd via bit-hack rsqrt + two Newton steps entirely on DVE: the
  Scalar engine keeps ONE activation table (exp/gelu boundary only)
  loaded; the Sqrt/Ln variants caused ~13 x 1.3us table reloads/layer
  that serialized the strict-FIFO Scalar queue against gelu/exp.
- residual+bias fused into one DVE scalar_tensor_tensor op.
- V head-split copies consolidated 12->2 per key chunk (strided AP).
- mean/sumsq stat matmuls packed into one PSUM bank (rows 0/32),
  freeing 2 PSUM banks for deeper matmul pipelining (psA/psB bufs=3).
- normalized-input buffers rotate between two tags so each LN finish
  overlaps the previous phase (LN2-h0 under attention-h1, next-layer
  LN1 under the FFN tail); final LN lands in x_sb for the head.
- QK matmuls (64-deep contraction) of a head pair issued to disjoint
  PE array row-groups via tile_position for in-array concurrency.
- causal masking free for fully-hidden key chunks (skipped), one 128x128
  triangular multiply for diagonal chunks.
Measured (probes): PE/DVE/Act streams and PE->DVE sync chains all match
the CoreSim cost model on this hardware; the residual HW-vs-sim gap
(~1.35x) tracks DMA/SBUF-port contention, not engine speed.

logits DRAM [12800, 1024] is vocab-major; host transposes.
"""

import os
import time

import numpy as np

L, D, H, V, T = 6, 768, 12, 50257, 1024
HD = D // H          # 64
B = 2
EPS = 1e-5
NCORES = 8
TPC = 1024           # tokens per core (the whole batch element)
DC = D // 128        # 6 feature chunks
FC = (4 * D) // 128  # 24 FFN chunks
VPAD = 51200         # padded vocab = 4 * 12800
VPC = VPAD // 4      # 12800 vocab columns per core (quarter)
VG = 20              # head weight streaming groups
VCG = VPC // VG      # 640
NKC = T // 128       # 8 key chunks
NL = int(os.environ.get("GPT_LAYERS", str(L)))
REPS = int(os.environ.get("GPT_REPS", "1"))

_RUNNER = None


def _build_nc():
    import concourse.mybir as mybir
    import concourse.tile as tile
    from concourse import bacc
    from concourse.alu_op_type import AluOpType

    f32 = mybir.dt.float32
    bf16 = mybir.dt.bfloat16
    AFT = mybir.ActivationFunctionType
    SCALE = float(1.0 / np.sqrt(HD))

    nc = bacc.Bacc(None, target_bir_lowering=False, num_devices=NCORES)

    x0T = nc.dram_tensor("x0T", [D, TPC], bf16, kind="ExternalInput")
    trimask = nc.dram_tensor("trimask", [128, 128], bf16, kind="ExternalInput")
    wq = nc.dram_tensor("wq", [NL, D, D], bf16, kind="ExternalInput")
    wk = nc.dram_tensor("wk", [NL, D, D], bf16, kind="ExternalInput")
    wv = nc.dram_tensor("wv", [NL, D, D], bf16, kind="ExternalInput")
    wo = nc.dram_tensor("wo", [NL, D, D], bf16, kind="ExternalInput")
    qb = nc.dram_tensor("qb", [NL, D], f32, kind="ExternalInput")
    kb = nc.dram_tensor("kb", [NL, D], f32, kind="ExternalInput")
    bo = nc.dram_tensor("bo", [NL, D], f32, kind="ExternalInput")
    w1 = nc.dram_tensor("w1", [NL, D, 4 * D], bf16, kind="ExternalInput")
    b1 = nc.dram_tensor("b1", [NL, 4 * D], f32, kind="ExternalInput")
    w2 = nc.dram_tensor("w2", [NL, 4 * D, D], bf16, kind="ExternalInput")
    b2 = nc.dram_tensor("b2", [NL, D], f32, kind="ExternalInput")
    whead = nc.dram_tensor("whead", [D, VPC], bf16, kind="ExternalInput")
    hb = nc.dram_tensor("hb", [VPC], f32, kind="ExternalInput")
    logits = nc.dram_tensor("logits", [VPC, TPC], bf16, kind="ExternalOutput")

    with tile.TileContext(nc, trace_sim=bool(int(os.environ.get(
            "GPT_TRACE_SIM", "0")))) as tc:
        with (
            tc.tile_pool(name="consts", bufs=1) as consts,
            tc.tile_pool(name="xpool", bufs=1) as xpool,
        ):
            # stat-MM lhsT: col 0 = ones (mean row), cols 1..32 = zeros.
            # Used as the start=True group opener it writes partitions 0..32
            # of the stat bank: row 0 = sum(x), row 32 zeroed for the sumsq
            # chain (matmul out base partition must be 0/32/64).
            ones33_bf = consts.tile([128, 33], bf16)
            nc.vector.memset(ones33_bf, 0.0)
            nc.vector.memset(ones33_bf[:, 0:1], 1.0)
            ones_bf = ones33_bf[:, 0:1]
            ones_row = consts.tile([1, 128], f32)      # bcast-MM lhsT
            nc.vector.memset(ones_row, 1.0)
            eps_t = consts.tile([1, 1], f32)
            nc.vector.memset(eps_t, EPS)
            tri_sb = consts.tile([128, 128], bf16)
            nc.sync.dma_start(out=tri_sb, in_=trimask[:, :])

            for _rep in range(REPS):
                x_sb = xpool.tile([128, DC, TPC], bf16)
                nc.sync.dma_start(
                    out=x_sb, in_=x0T.rearrange("(c p) q -> p c q", p=128))

                with (
                    tc.tile_pool(name="wa", bufs=2) as wa,
                    tc.tile_pool(name="wffn", bufs=1) as wffn,
                    tc.tile_pool(name="acts", bufs=1) as acts,
                    tc.tile_pool(name="tmp", bufs=2) as tmp,
                    tc.tile_pool(name="rows", bufs=1) as rows,
                    tc.tile_pool(name="attn", bufs=2) as attn,
                    tc.tile_pool(name="bcp", bufs=4) as bcp,
                    tc.tile_pool(name="psA", bufs=3, space="PSUM") as psA,
                    tc.tile_pool(name="psB", bufs=3, space="PSUM") as psB,
                    tc.tile_pool(name="psS", bufs=2, space="PSUM") as psS,
                ):
                    def ln_stats(hf):
                        """Accumulate [sum(x) row 0; sum(x^2) row 32] for
                        token half hf into one PSUM bank."""
                        cs = slice(512 * hf, 512 * (hf + 1))
                        st_ps = psS.tile([33, 512], f32, tag="stat")
                        # opener: row0 += sum x, rows 1..32 zeroed
                        nc.tensor.matmul(st_ps, ones33_bf, x_sb[:, 0, cs],
                                         start=True, stop=False,
                                         skip_group_check=True)
                        for c in range(1, DC):
                            nc.tensor.matmul(st_ps[0:1, :], ones_bf,
                                             x_sb[:, c, cs],
                                             start=False, stop=(c == DC - 1),
                                             skip_group_check=True)
                        for c in range(DC):
                            sqb = tmp.tile([128, 512], bf16, tag="sqb")
                            nc.vector.tensor_mul(sqb, x_sb[:, c, cs],
                                                 x_sb[:, c, cs])
                            nc.tensor.matmul(st_ps[32:33, :], ones_bf, sqb,
                                             start=False, stop=(c == DC - 1),
                                             skip_group_check=True)
                        return st_ps

                    def ln_finish(st_ps, out_tile, hf):
                        """out = (x - mean) * rstd for token half hf.
                        Scale/shift are folded into downstream weights.
                        rstd comes from the bit-hack rsqrt + one Newton
                        step, entirely on DVE: keeps the Scalar engine's
                        single activation table (exp/tanh/square/identity)
                        loaded for the whole kernel."""
                        cs = slice(512 * hf, 512 * (hf + 1))
                        m_row = rows.tile([1, 512], f32, tag="mrow", bufs=1)
                        nc.vector.tensor_scalar_mul(m_row, st_ps[0:1, :],
                                                    1.0 / D)
                        var = rows.tile([1, 512], f32, tag="var", bufs=1)
                        nc.vector.tensor_scalar_mul(var, st_ps[32:33, :],
                                                    1.0 / D)
                        msq = rows.tile([1, 512], f32, tag="msq", bufs=1)
                        nc.vector.tensor_mul(msq, m_row, m_row)
                        nc.vector.tensor_sub(var, var, msq)
                        nc.vector.tensor_scalar_add(var, var, EPS)
                        yi = rows.tile([1, 512], mybir.dt.int32, tag="yi",
                                       bufs=1)
                        nc.vector.tensor_scalar(
                            out=yi, in0=var.bitcast(mybir.dt.int32),
                            scalar1=1, scalar2=None,
                            op0=AluOpType.logical_shift_right)
                        nc.vector.tensor_scalar(
                            out=yi, in0=yi, scalar1=-1, scalar2=0x5F3759DF,
                            op0=AluOpType.mult, op1=AluOpType.add)
                        y = yi.bitcast(f32)
                        nt = rows.tile([1, 512], f32, tag="nt", bufs=1)
                        nc.vector.tensor_mul(nt, y, y)
                        nc.vector.tensor_mul(nt, nt, var)
                        nc.vector.tensor_scalar(
                            out=nt, in0=nt, scalar1=-0.5, scalar2=1.5,
                            op0=AluOpType.mult, op1=AluOpType.add)
                        # second Newton step in two ops keeps full f32 y
                        nc.vector.tensor_mul(y, y, nt)
                        nc.vector.tensor_mul(nt, y, y)
                        nc.vector.tensor_mul(nt, nt, var)
                        nc.vector.tensor_scalar(
                            out=nt, in0=nt, scalar1=-0.5, scalar2=1.5,
                            op0=AluOpType.mult, op1=AluOpType.add)
                        rstd = rows.tile([1, 512], bf16, tag="rstd", bufs=1)
                        nc.vector.tensor_mul(rstd, y, nt)
                        mrs = rows.tile([1, 512], bf16, tag="mrs", bufs=1)
                        nc.vector.tensor_mul(mrs, m_row, rstd)
                        # broadcast the per-token rows across partitions on
                        # the (otherwise idle) GPSIMD engine, into SBUF so
                        # the LN muls below run in the fast 16-bit DVE mode
                        bcA = bcp.tile([128, 512], bf16, tag="bc")
                        bcB = bcp.tile([128, 512], bf16, tag="bc")
                        nc.gpsimd.partition_broadcast(bcA, rstd)
                        nc.gpsimd.partition_broadcast(bcB, mrs)
                        for c in range(DC):
                            t1 = tmp.tile([128, 512], bf16, tag="lnt")
                            nc.vector.tensor_mul(t1, x_sb[:, c, cs], bcA)
                            nc.vector.tensor_sub(out_tile[:, c, cs], t1, bcB)

                    def pcol(t_sb):
                        return lambda c: t_sb[:, c:c + 1]

                    def load_pcol(pool_tag, src, li):
                        t_sb = acts.tile([128, src.shape[-1] // 128], f32,
                                         tag=pool_tag)
                        ap = src[li] if len(src.shape) == 2 else src
                        nc.sync.dma_start(
                            out=t_sb, in_=ap.rearrange("(c p) -> p c", p=128))
                        return t_sb

                    # layer 0 LN1 (startup). The normalized-input buffers
                    # rotate between two tags so each layer's LN finish can
                    # overlap the previous phase's compute.
                    h_cur = acts.tile([128, DC, TPC], bf16, tag="hA")
                    for hf in range(2):
                        ln_finish(ln_stats(hf), h_cur, hf)

                    for li in range(NL):
                        tB = "hB" if li % 2 == 0 else "hA"
                        wk_sb = wa.tile([128, DC, D], bf16, tag="wa")
                        nc.sync.dma_start(
                            out=wk_sb,
                            in_=wk[li].rearrange("(c p) n -> p c n", p=128))
                        wv_sb = wa.tile([128, DC, D], bf16, tag="wa")
                        nc.sync.dma_start(
                            out=wv_sb,
                            in_=wv[li].rearrange("(c p) n -> p c n", p=128))
                        w1_sb = wffn.tile([128, DC, 4 * D], bf16, tag="w1")
                        nc.sync.dma_start(
                            out=w1_sb,
                            in_=w1[li].rearrange("(c p) n -> p c n", p=128))
                        w2_sb = wffn.tile([128, FC, D], bf16, tag="w2")
                        nc.sync.dma_start(
                            out=w2_sb,
                            in_=w2[li].rearrange("(c p) n -> p c n", p=128))
                        kb_sb = load_pcol("kb", kb, li)
                        qb_sb = load_pcol("qb", qb, li)
                        bo_sb = load_pcol("bo", bo, li)
                        b2_sb = load_pcol("b2", b2, li)
                        b1_sb = load_pcol("b1", b1, li)

                        kT_sb = acts.tile([128, DC, TPC], bf16, tag="kT")
                        vext = []
                        for kc in range(NKC):
                            vt = attn.tile([128, H, HD + 1], bf16, tag="vext",
                                           bufs=8)
                            nc.vector.memset(vt[:, :, HD:HD + 1], 1.0)
                            vext.append(vt)

                        # K/V projections, pipelined by half (h_cur was
                        # finished in the previous layer's FFN tail)
                        for hf in range(2):
                            cs = slice(512 * hf, 512 * (hf + 1))
                            for oc in range(DC):
                                ps = psA.tile([128, 512], f32, tag="lin")
                                for c in range(DC):
                                    nc.tensor.matmul(
                                        ps,
                                        wk_sb[:, c, 128 * oc:128 * (oc + 1)],
                                        h_cur[:, c, cs],
                                        start=(c == 0), stop=(c == DC - 1))
                                nc.scalar.activation(
                                    out=kT_sb[:, oc, cs], in_=ps,
                                    func=AFT.Identity,
                                    bias=kb_sb[:, oc:oc + 1], scale=1.0)
                            for kc in range(4 * hf, 4 * (hf + 1)):
                                for nh in range(2):
                                    ps = psA.tile([128, 6, HD], f32,
                                                  tag="lin")
                                    for c in range(DC):
                                        nc.tensor.matmul(
                                            ps,
                                            h_cur[:, c,
                                                  128 * kc:128 * (kc + 1)],
                                            wv_sb[:, c,
                                                  384 * nh:384 * (nh + 1)],
                                            start=(c == 0),
                                            stop=(c == DC - 1))
                                    nc.scalar.activation(
                                        out=vext[kc][:, 6 * nh:6 * nh + 6,
                                                     0:HD],
                                        in_=ps, func=AFT.Copy)

                        # Q projection
                        wq_sb = wa.tile([128, DC, D], bf16, tag="wa")
                        nc.sync.dma_start(
                            out=wq_sb,
                            in_=wq[li].rearrange("(c p) n -> p c n", p=128))
                        qT_sb = acts.tile([128, DC, TPC], bf16, tag="qT")
                        for hf in range(2):
                            cs = slice(512 * hf, 512 * (hf + 1))
                            for oc in range(DC):
                                ps = psA.tile([128, 512], f32, tag="lin")
                                for c in range(DC):
                                    nc.tensor.matmul(
                                        ps,
                                        wq_sb[:, c, 128 * oc:128 * (oc + 1)],
                                        h_cur[:, c, cs],
                                        start=(c == 0), stop=(c == DC - 1))
                                nc.scalar.activation(
                                    out=qT_sb[:, oc, cs], in_=ps,
                                    func=AFT.Identity,
                                    bias=qb_sb[:, oc:oc + 1], scale=1.0)

                        wo_sb = wa.tile([128, DC, D], bf16, tag="wa")
                        nc.sync.dma_start(
                            out=wo_sb,
                            in_=wo[li].rearrange("(c p) n -> p c n", p=128))

                        # attention halves; out-proj + LN2 pipelined under
                        # the opposite half / the FFN. h2 reuses h_cur's
                        # buffer (dead after the Q projection); ctx gets the
                        # other rotation buffer.
                        ctx_sb = acts.tile([128, DC, TPC], bf16, tag=tB)
                        h2_sb = acts.tile([128, DC, TPC], bf16,
                                          tag="hA" if li % 2 == 0 else "hB")
                        for hf in range(2):
                            nkc = 4 * (hf + 1)
                            for hp in range(H // 2):
                                pair = (2 * hp, 2 * hp + 1)
                                ctxps = {}
                                for h in pair:
                                    ct = psB.tile([HD + 1, 512], f32,
                                                  tag="ctx")
                                    ctxps[h] = ct
                                for kc in range(nkc):
                                    c0 = max(0, 128 * kc - 512 * hf)
                                    ets = {}
                                    for h in pair:
                                        pb = 64 * (h % 2)
                                        s_ps = psA.tile([128, 512], f32,
                                                        tag="lin")
                                        # the pair's two 64-deep QK matmuls
                                        # go to disjoint array row-groups so
                                        # they run concurrently in the PE
                                        nc.tensor.matmul(
                                            s_ps[:, c0:512],
                                            kT_sb[pb:pb + 64, h // 2,
                                                  128 * kc:128 * (kc + 1)],
                                            qT_sb[pb:pb + 64, h // 2,
                                                  512 * hf + c0:
                                                  512 * (hf + 1)],
                                            start=True, stop=True,
                                            tile_position=(pb, 0))
                                        e_t = tmp.tile([128, 512], bf16,
                                                       tag="et", bufs=4)
                                        nc.scalar.activation(
                                            out=e_t[:, c0:512],
                                            in_=s_ps[:, c0:512],
                                            func=AFT.Exp, scale=SCALE)
                                        if 128 * kc >= 512 * hf:
                                            nc.vector.tensor_mul(
                                                e_t[:, c0:c0 + 128],
                                                e_t[:, c0:c0 + 128], tri_sb)
                                        ets[h] = e_t
                                    for h in pair:
                                        nc.tensor.matmul(
                                            ctxps[h][:, c0:512],
                                            vext[kc][:, h, :],
                                            ets[h][:, c0:512],
                                            start=(kc == 0),
                                            stop=(kc == nkc - 1))
                                for h in pair:
                                    pb = 64 * (h % 2)
                                    ctx_ps = ctxps[h]
                                    r_row = rows.tile([1, 512], f32,
                                                      tag="rrow", bufs=2)
                                    nc.vector.reciprocal(r_row,
                                                         ctx_ps[HD:HD + 1, :])
                                    bc_ps = psA.tile([64, 512], f32,
                                                     tag="lin")
                                    nc.tensor.matmul(bc_ps, ones_row[:, 0:64],
                                                     r_row, start=True,
                                                     stop=True)
                                    dst = ctx_sb[pb:pb + 64, h // 2,
                                                 512 * hf:512 * (hf + 1)]
                                    nc.vector.tensor_copy(dst,
                                                          ctx_ps[0:HD, :])
                                    nc.vector.tensor_mul(dst, dst, bc_ps)

                            cs = slice(512 * hf, 512 * (hf + 1))
                            for oc in range(DC):
                                ps = psA.tile([128, 512], f32, tag="lin")
                                for c in range(DC):
                                    nc.tensor.matmul(
                                        ps,
                                        wo_sb[:, c, 128 * oc:128 * (oc + 1)],
                                        ctx_sb[:, c, cs],
                                        start=(c == 0), stop=(c == DC - 1))
                                nc.vector.scalar_tensor_tensor(
                                    out=x_sb[:, oc, cs], in0=ps,
                                    scalar=bo_sb[:, oc:oc + 1],
                                    in1=x_sb[:, oc, cs],
                                    op0=AluOpType.add, op1=AluOpType.add)
                            ln_finish(ln_stats(hf), h2_sb, hf)

                        # FFN in token quarters; the next layer's LN1
                        # (or the final LN) finishes in the FFN tail into
                        # the rotation buffer freed by ctx (the final LN
                        # goes in place into x_sb so the head can read it
                        # after the layer pools close)
                        last = li == NL - 1
                        h_next = x_sb if last else acts.tile(
                            [128, DC, TPC], bf16, tag=tB)
                        for tq in range(4):
                            cs = slice(256 * tq, 256 * (tq + 1))
                            h1_sb = acts.tile([128, FC, 256], bf16, tag="h1")
                            for fc in range(FC):
                                ps = psA.tile([128, 256], f32, tag="lin")
                                for c in range(DC):
                                    nc.tensor.matmul(
                                        ps,
                                        w1_sb[:, c, 128 * fc:128 * (fc + 1)],
                                        h2_sb[:, c, cs],
                                        start=(c == 0), stop=(c == DC - 1))
                                nc.scalar.activation(
                                    out=h1_sb[:, fc, :], in_=ps,
                                    func=AFT.Gelu_apprx_tanh,
                                    bias=b1_sb[:, fc:fc + 1], scale=1.0)
                            for oc in range(DC):
                                ps = psA.tile([128, 256], f32, tag="lin")
                                for fc in range(FC):
                                    nc.tensor.matmul(
                                        ps,
                                        w2_sb[:, fc, 128 * oc:128 * (oc + 1)],
                                        h1_sb[:, fc, :],
                                        start=(fc == 0), stop=(fc == FC - 1))
                                nc.vector.scalar_tensor_tensor(
                                    out=x_sb[:, oc, cs], in0=ps,
                                    scalar=b2_sb[:, oc:oc + 1],
                                    in1=x_sb[:, oc, cs],
                                    op0=AluOpType.add, op1=AluOpType.add)
                            if tq == 1:
                                tail_st = ln_stats(0)
                            elif tq == 3:
                                # both finishes after the last gelu so the
                                # Ln/Exp ops sit at the single gelu->exp
                                # act-table boundary instead of forcing a
                                # reload round-trip mid-FFN
                                ln_finish(tail_st, h_next, 0)
                                ln_finish(ln_stats(1), h_next, 1)
                        h_cur = h_next
                    # after the last layer xf_sb holds the final-LN output

                # LM head: this core's vocab quarter for its batch
                with (
                    tc.tile_pool(name="whp", bufs=2) as whp,
                    tc.tile_pool(name="hbp", bufs=1) as hbp,
                    tc.tile_pool(name="hout", bufs=3) as hout,
                    tc.tile_pool(name="psH", bufs=3, space="PSUM") as psH,
                ):
                    hb_sb = hbp.tile([128, VPC // 128], f32, tag="hb")
                    nc.sync.dma_start(
                        out=hb_sb, in_=hb.rearrange("(c p) -> p c", p=128))
                    for g in range(VG):
                        wh_sb = whp.tile([128, DC, VCG], bf16, tag="wh")
                        nc.sync.dma_start(
                            out=wh_sb,
                            in_=whead[:, g * VCG:(g + 1) * VCG]
                            .rearrange("(c p) n -> p c n", p=128))
                        for cc in range(VCG // 128):
                            vcol = g * (VCG // 128) + cc
                            for n in range(2):
                                ps = psH.tile([128, 512], f32, tag="hps")
                                for c in range(DC):
                                    nc.tensor.matmul(
                                        ps,
                                        wh_sb[:, c, 128 * cc:128 * (cc + 1)],
                                        x_sb[:, c, 512 * n:512 * (n + 1)],
                                        start=(c == 0), stop=(c == DC - 1))
                                ot = hout.tile([128, 512], bf16, tag="hout")
                                if (cc + n) % 2 == 0:
                                    nc.vector.tensor_scalar_add(
                                        ot, ps, hb_sb[:, vcol:vcol + 1])
                                else:
                                    nc.scalar.activation(
                                        out=ot, in_=ps, func=AFT.Identity,
                                        bias=hb_sb[:, vcol:vcol + 1],
                                        scale=1.0)
                                nc.sync.dma_start(
                                    out=logits[g * VCG + 128 * cc:
                                               g * VCG + 128 * (cc + 1),
                                               512 * n:512 * (n + 1)],
                                    in_=ot)
    nc.compile()
    return nc


# ---------------------------------------------------------------- host side

def _prep_in_maps(in_idx, tok_emb, pos_emb, Wq, Wk, Wv, Wo, bo, W1, b1, W2, b2,
                  ln1_s, ln1_b, ln2_s, ln2_b, fn_s, fn_b, W_head):
    import ml_dtypes
    bf = ml_dtypes.bfloat16
    f32 = np.float32

    in_idx = np.asarray(in_idx)
    tok_emb = np.asarray(tok_emb, dtype=f32)
    pos_emb = np.asarray(pos_emb, dtype=f32)
    x0 = tok_emb[in_idx] + pos_emb[None, :T]          # [B, T, D] f32

    Wq = np.asarray(Wq, f32)[:NL]
    Wk = np.asarray(Wk, f32)[:NL]
    Wv = np.asarray(Wv, f32)[:NL]
    Wo = np.asarray(Wo, f32)[:NL]
    W1 = np.asarray(W1, f32)[:NL]
    W2 = np.asarray(W2, f32)[:NL]
    bo = np.asarray(bo, f32)[:NL]
    b1 = np.asarray(b1, f32)[:NL]
    b2 = np.asarray(b2, f32)[:NL]
    ln1_s = np.asarray(ln1_s, f32)[:NL]
    ln1_b = np.asarray(ln1_b, f32)[:NL]
    ln2_s = np.asarray(ln2_s, f32)[:NL]
    ln2_b = np.asarray(ln2_b, f32)[:NL]
    fn_s = np.asarray(fn_s, f32)
    fn_b = np.asarray(fn_b, f32)
    W_head = np.asarray(W_head, f32)

    # Fold LN scales into the consuming weights, LN shifts into biases.
    wq_b = (ln1_s[:, :, None] * Wq).astype(bf)
    wk_b = (ln1_s[:, :, None] * Wk).astype(bf)
    wv_b = (ln1_s[:, :, None] * Wv).astype(bf)
    wo_b = Wo.astype(bf)
    w1_b = (ln2_s[:, :, None] * W1).astype(bf)
    w2_b = W2.astype(bf)
    qb_f = np.einsum('ld,ldo->lo', ln1_b, Wq).astype(f32)
    kb_f = np.einsum('ld,ldo->lo', ln1_b, Wk).astype(f32)
    vb = np.einsum('ld,ldo->lo', ln1_b, Wv)
    # softmax rows sum to 1, so a constant V offset passes straight
    # through attention into the out-proj input.
    bo_f = (bo + np.einsum('ld,ldo->lo', vb, Wo)).astype(f32)
    b1_f = (b1 + np.einsum('ld,ldf->lf', ln2_b, W1)).astype(f32)
    b2_f = np.ascontiguousarray(b2)
    wh_scaled = fn_s[:, None] * W_head
    hb_full = np.zeros(VPAD, f32)
    hb_full[:V] = fn_b @ W_head
    wh_pad = np.zeros((D, VPAD), bf)
    wh_pad[:, :V] = wh_scaled.astype(bf)
    tri = np.triu(np.ones((128, 128), f32)).astype(bf)   # 1 where k <= q

    in_maps = []
    for r in range(NCORES):
        b, vq = divmod(r, 4)
        x0T = np.ascontiguousarray(x0[b].T).astype(bf)     # [768, 1024] bf16
        whead_r = np.ascontiguousarray(wh_pad[:, vq * VPC:(vq + 1) * VPC])
        hb_r = np.ascontiguousarray(hb_full[vq * VPC:(vq + 1) * VPC])
        in_maps.append({
            "x0T": x0T, "trimask": tri,
            "wq": wq_b, "wk": wk_b, "wv": wv_b, "wo": wo_b,
            "qb": qb_f, "kb": kb_f, "bo": bo_f,
            "w1": w1_b, "b1": b1_f, "w2": w2_b, "b2": b2_f,
            "whead": whead_r, "hb": hb_r,
        })
    return in_maps


class _Runner:
    """Builds the Bass program once and a reusable sharded-jit executable."""

    def __init__(self):
        import jax
        import concourse.mybir as mybir
        from concourse import bass2jax
        from jax.sharding import Mesh, PartitionSpec
        from jax.experimental.shard_map import shard_map

        bass2jax.install_neuronx_cc_hook()
        nc = _build_nc()
        self.nc = nc
        _bass_exec_p = bass2jax._bass_exec_p

        partition_name = (nc.partition_id_tensor.name
                          if nc.partition_id_tensor else None)
        in_names, out_names, out_avals, zero_outs = [], [], [], []
        for alloc in nc.m.functions[0].allocations:
            if not isinstance(alloc, mybir.MemoryLocationSet):
                continue
            name = alloc.memorylocations[0].name
            if alloc.kind == "ExternalInput":
                if name != partition_name:
                    in_names.append(name)
            elif alloc.kind == "ExternalOutput":
                out_names.append(name)
                shape = tuple(alloc.tensor_shape)
                dtype = mybir.dt.np(alloc.dtype)
                out_avals.append(jax.core.ShapedArray(shape, dtype))
                zero_outs.append(np.zeros(shape, dtype))
        n_params = len(in_names)
        all_in_names = list(in_names) + list(out_names)
        if partition_name is not None:
            all_in_names.append(partition_name)

        def _body(*args):
            operands = list(args)
            if partition_name is not None:
                operands.append(bass2jax.partition_id_tensor())
            outs = _bass_exec_p.bind(
                *operands,
                out_avals=tuple(out_avals),
                in_names=tuple(all_in_names),
                out_names=tuple(out_names),
                lowering_input_output_aliases=(),
                sim_require_finite=True,
                sim_require_nnan=True,
                nc=nc,
            )
            return tuple(outs)

        devices = jax.devices()[:NCORES]
        mesh = Mesh(np.asarray(devices), ("core",))
        n_outs = len(out_names)
        in_specs = (PartitionSpec("core"),) * (n_params + n_outs)
        out_specs = (PartitionSpec("core"),) * n_outs
        self.sharded = jax.jit(
            shard_map(_body, mesh=mesh, in_specs=in_specs,
                      out_specs=out_specs, check_rep=False),
            keep_unused=True,
        )
        self.jax = jax
        self.mesh = mesh
        self.in_names = in_names
        self.out_names = out_names
        self.zero_outs = zero_outs
        self.n_params = n_params
        self.out_avals = out_avals

    def stage(self, in_maps):
        """Concatenate per-core inputs and move them to the devices."""
        import jax
        from jax.sharding import NamedSharding, PartitionSpec
        sh = NamedSharding(self.mesh, PartitionSpec("core"))
        concat = [np.concatenate([in_maps[c][n] for c in range(NCORES)], axis=0)
                  for n in self.in_names]
        concat += [np.concatenate([z] * NCORES, axis=0) for z in self.zero_outs]
        return [jax.device_put(a, sh) for a in concat]

    def run(self, staged):
        outs = self.sharded(*staged)
        outs = [o.block_until_ready() for o in outs]
        return outs

    def collect(self, outs):
        res = []
        for c in range(NCORES):
            d = {}
            for i, name in enumerate(self.out_names):
                shp = self.out_avals[i].shape
                d[name] = np.asarray(outs[i]).reshape(NCORES, *shp)[c]
            res.append(d)
        return res


def _get_runner():
    global _RUNNER
    if _RUNNER is None:
        _RUNNER = _Runner()
    return _RUNNER


def _assemble(results):
    """Per-core [12800, 1024] bf16 vocab-major -> [B, T, V] f32."""
    out = np.empty((B, T, V), np.float32)
    for b in range(B):
        big = np.concatenate(
            [results[4 * b + vq]["logits"] for vq in range(4)], axis=0)
        out[b] = big.T.astype(np.float32)[:, :V]
    return out


def kernel(**inputs):
    r = _get_runner()
    staged = r.stage(_prep_in_maps(**inputs))
    outs = r.run(staged)
    return _assemble(r.collect(outs))


def run_timed(inputs, iters=3):
    """Returns (full_output, best_wall_ns_of_warm_execute)."""
    r = _get_runner()
    staged = r.stage(_prep_in_maps(**inputs))
    outs = r.run(staged)          # warmup (includes compile on first use)
    best = None
    for _ in range(iters):
        t0 = time.perf_counter()
        r.run(staged)
        dt = time.perf_counter() - t0
        best = dt if best is None or dt < best else best
    return _assemble(r.collect(outs)), int(best * 1e9)


def run_timed_diff(inputs, r_hi=5, iters=8):
    """Measure true on-device time of one forward pass by differencing a
    1-repetition NEFF against an r_hi-repetition NEFF in the same process
    (the ~80 ms PJRT/axon dispatch floor cancels out; no NTFF profiling is
    available under this axon client). Returns (full_output, per_pass_ns).
    """
    global REPS
    in_maps = _prep_in_maps(**inputs)
    old = REPS
    runners = {}
    for rr in (1, r_hi):
        REPS = rr
        runners[rr] = _Runner()
    REPS = old
    staged = {rr: runners[rr].stage(in_maps) for rr in runners}
    outs = runners[1].run(staged[1])
    runners[r_hi].run(staged[r_hi])
    lo, hi = [], []
    for _ in range(iters):
        t0 = time.perf_counter()
        runners[r_hi].run(staged[r_hi])
        hi.append(time.perf_counter() - t0)
        t0 = time.perf_counter()
        runners[1].run(staged[1])
        lo.append(time.perf_counter() - t0)
    lo.sort()
    hi.sort()
    d = (hi[len(hi) // 2] - lo[len(lo) // 2]) / (r_hi - 1)
    return _assemble(runners[1].collect(outs)), max(int(d * 1e9), 0)
